# revision 14
# baseline (speedup 1.0000x reference)
"""Batch Graph VAE (GCN + DiffPool encoder, DiffUnpool decoder) on 8 TRN2 NeuronCores.

Sharding: data-parallel over batch (B=8 -> 1 sample/core); adj + weights replicated.

Per-core math (restructured with matmul associativity; adj/A1/A2 symmetric so they
are used directly as the TensorE stationary operand lhsT):
  fmAx = (adj @ x)^T                  h1 = relu(Ax @ We1 + be1)
  Ah1  = adj @ h1                     s1 = softmax(Ah1 @ Kpool1)
  p1   = (s1^T Ah1) @ Kemb1           A1 = s1^T (adj @ s1)
  h2   = relu(A1 @ (p1 We2) + be2)    Ah2 = A1 @ h2
  s2   = softmax(Ah2 @ Kpool2)        p2 = (s2^T Ah2) @ Kemb2
  A2   = s2^T (A1 @ s2)               h3 = A2 @ (p2 We3) + be3
  mean, lv = split(h3); z = mean + exp(.5 lv) eps
  d0   = relu((A2 @ z) @ Wd0 + bd0)   u1 = s2 @ d0
  d1   = relu(A1 @ (u1 Wd1) + bd1)    u2 = s1 @ d1
  d2   = relu(adj @ (u2 Wd2) + bd2)   out = softplus(adj @ (d2 Wdf) + bdf)

All matmuls in bf16 with fp32 PSUM accumulation (measured 6.7e-3 rel err vs fp32
reference on the host; fp32 everywhere would be 4x slower on the PE).
Feature-major ("fm") copies needed for X@W products come from DMA-XBAR transposes
(bf16 128x128 tiles) so the PE does no transpose work. SBUF slot groups share
tags (serial lifetimes) to fit the ~208KB/partition budget.
"""

import numpy as np
import ml_dtypes

import concourse.bass as bass
import concourse.mybir as mybir
import concourse.tile as tile
from concourse import bacc

dt = mybir.dt
F32 = dt.float32
BF16 = dt.bfloat16
AF = mybir.ActivationFunctionType
ALU = mybir.AluOpType
AX = mybir.AxisListType

P = 128
N0, N1, N2 = 2048, 1024, 512
F, H, LAT = 64, 256, 64
NCORES = 8

_CACHED_NC = None


def _r3(d):
    # DRAM [nodes, f] -> [p, nodes/P, f] with node = o*P + p
    return d[:].rearrange("(o p) f -> p o f", p=P)


def _build(chain=1):
    nc = bacc.Bacc(trn_type="TRN2", target_bir_lowering=False)

    # ---- DRAM I/O ----
    x_d = nc.dram_tensor("x", [N0, F], BF16, kind="ExternalInput")
    eps_d = nc.dram_tensor("eps", [N2, LAT], F32, kind="ExternalInput")
    adj_d = nc.dram_tensor("adj", [N0, N0], BF16, kind="ExternalInput")
    W = {}
    for name, shape in [
        ("We1", [F, H]), ("Kemb1", [H, H]), ("Kpool1", [H, N1]),
        ("We2", [H, H]), ("Kemb2", [H, H]), ("Kpool2", [H, N2]),
        ("We3", [H, 2 * LAT]), ("Wd0", [LAT, H]), ("Wd1", [H, H]),
        ("Wd2", [H, H]), ("Wdf", [H, F]),
    ]:
        W[name] = nc.dram_tensor(name, shape, BF16, kind="ExternalInput")
    Bs = {}
    for name, n in [
        ("be1", H), ("be2", H), ("be3", 2 * LAT),
        ("bd0", H), ("bd1", H), ("bd2", H), ("bdf", F),
    ]:
        Bs[name] = nc.dram_tensor(name, [n], F32, kind="ExternalInput")
    out_d = nc.dram_tensor("out", [N0, F], F32, kind="ExternalOutput")
    mean_d = nc.dram_tensor("mean", [N2, LAT], F32, kind="ExternalOutput")
    lv_d = nc.dram_tensor("log_var", [N2, LAT], F32, kind="ExternalOutput")

    with tile.TileContext(nc) as tc:
        _emit(nc, tc, x_d, eps_d, adj_d, W, Bs, out_d, mean_d, lv_d, chain)

    nc.finalize()
    return nc


def _emit(nc, tc, x_d, eps_d, adj_d, W, Bs, out_d, mean_d, lv_d, chain=1):
    main = tc.alloc_tile_pool(name="main", bufs=1)
    psum = tc.alloc_tile_pool(name="psum", bufs=6, space="PSUM")
    smx = tc.alloc_tile_pool(name="smx", bufs=2)
    evq = [0]

    # ---------- constants / weights ----------
    adj_sb = main.tile([P, N0 // P, N0], BF16, tag="adj")
    adj_r = _r3(adj_d)
    for o in range(N0 // P):
        nc.sync.dma_start(out=adj_sb[:, o, :], in_=adj_r[:, o, :])

    def load_w(name, k, n):
        t = main.tile([P, k // P, n], BF16, tag=name)
        nc.sync.dma_start(out=t[:], in_=_r3(W[name]))
        return t

    def load_w_pad(name, k, n):
        # contraction dim k < 128: zero-pad partitions [k:128]
        t = main.tile([P, 1, n], BF16, tag=name)
        nc.vector.memset(t[:], 0.0)
        nc.sync.dma_start(out=t[0:k, 0, :], in_=W[name][:, :])
        return t

    We1 = load_w_pad("We1", F, H)
    Kemb1 = load_w("Kemb1", H, H)
    Kpool1 = load_w("Kpool1", H, N1)
    We2 = load_w("We2", H, H)
    Kemb2 = load_w("Kemb2", H, H)
    Kpool2 = load_w("Kpool2", H, N2)
    We3 = load_w("We3", H, 2 * LAT)
    Wd0 = load_w_pad("Wd0", LAT, H)
    Wd1 = load_w("Wd1", H, H)
    Wd2 = load_w("Wd2", H, H)
    Wdf = load_w("Wdf", H, F)

    def load_bias(name, n):
        t = main.tile([P, n], F32, tag=name)
        src = Bs[name][:]
        bcast = bass.AP(tensor=src.tensor, offset=src.offset,
                        ap=[[0, P], list(src.ap[0])])
        nc.gpsimd.dma_start(out=t[:], in_=bcast)
        return t

    be1 = load_bias("be1", H)
    be2 = load_bias("be2", H)
    be3 = load_bias("be3", 2 * LAT)
    bd0 = load_bias("bd0", H)
    bd1 = load_bias("bd1", H)
    bd2 = load_bias("bd2", H)
    bdf = load_bias("bdf", F)

    eps_sb = main.tile([P, N2 // P, LAT], F32, tag="eps")
    nc.sync.dma_start(out=eps_sb[:], in_=_r3(eps_d))

    x_sb = main.tile([P, N0 // P, F], BF16, tag="x")
    nc.sync.dma_start(out=x_sb[:], in_=_r3(x_d))

    # ---------- helpers ----------
    def lhsT_of(t):
        return lambda ki, m: t[:, ki, m * P:(m + 1) * P]

    def rhs_of(t, off=0):
        return lambda ki, n0, w: t[:, ki, off + n0:off + n0 + w]

    def gemm(M, K, N, lhsT_fn, rhs_fn, writer):
        mt, kt = M // P, (K + P - 1) // P
        for m in range(mt):
            for n0 in range(0, N, 512):
                w = min(512, N - n0)
                ps = psum.tile([P, 512], F32, tag="ps")
                for ki in range(kt):
                    nc.tensor.matmul(ps[:, 0:w], lhsT_fn(ki, m),
                                     rhs_fn(ki, n0, w),
                                     start=(ki == 0), stop=(ki == kt - 1))
                writer(m, n0, w, ps)

    def copy_evict(dst_ap, src_ap):
        if evq[0] % 2 == 0:
            nc.vector.tensor_copy(out=dst_ap, in_=src_ap)
        else:
            nc.scalar.copy(out=dst_ap, in_=src_ap)
        evq[0] += 1

    def w_copy(dst):
        def wr(m, n0, w, ps):
            copy_evict(dst[:, m, n0:n0 + w], ps[:, 0:w])
        return wr

    def w_copy_off(dst, off):
        def wr(m, n0, w, ps):
            copy_evict(dst[:, m, off + n0:off + n0 + w], ps[:, 0:w])
        return wr

    def w_bias_act(dst, bias_t, act):
        def wr(m, n0, w, ps):
            o = dst[:, m, n0:n0 + w]
            nc.vector.tensor_tensor(o, ps[:, 0:w], bias_t[:, n0:n0 + w], ALU.add)
            if act is not None:
                nc.scalar.activation(o, o, act)
        return wr

    def fm_transpose(dst_fm, src_nat, nodes, feat):
        # DMA-XBAR transpose of bf16 128x128 tiles:
        # src_nat [P, nodes/P, feat] -> dst_fm [P, feat/P, nodes]
        for c in range(feat // P):
            for r in range(nodes // P):
                nc.sync.dma_start(out=dst_fm[:, c, r * P:(r + 1) * P],
                                  in_=src_nat[:, r, c * P:(c + 1) * P],
                                  transpose=True)

    def softmax_rows(dst, m, N, lhsT_fn, rhs_fn, kt):
        # pre-activation row-tile m of shape [P, N]; softmax over free axis
        nch = (N + 511) // 512
        pss = []
        for n0 in range(0, N, 512):
            ps = psum.tile([P, 512], F32, tag="ps")
            for ki in range(kt):
                nc.tensor.matmul(ps[:, :], lhsT_fn(ki, m), rhs_fn(ki, n0, 512),
                                 start=(ki == 0), stop=(ki == kt - 1))
            pss.append(ps)
        sc = smx.tile([P, 8], F32, tag="sstat")
        ex = smx.tile([P, N], BF16, tag="sexp")
        for i, ps in enumerate(pss):
            nc.vector.reduce_max(sc[:, i:i + 1], ps[:, :], axis=AX.X)
        if nch == 2:
            nc.vector.tensor_tensor(sc[:, 2:3], sc[:, 0:1], sc[:, 1:2], ALU.max)
            mx = sc[:, 2:3]
        else:
            mx = sc[:, 0:1]
        nc.vector.tensor_scalar_mul(sc[:, 3:4], mx, -1.0)
        for i, ps in enumerate(pss):
            nc.scalar.activation(ex[:, i * 512:(i + 1) * 512], ps[:, :], AF.Exp,
                                 bias=sc[:, 3:4], accum_out=sc[:, 4 + i:5 + i])
        if nch == 2:
            nc.vector.tensor_tensor(sc[:, 6:7], sc[:, 4:5], sc[:, 5:6], ALU.add)
            tot = sc[:, 6:7]
        else:
            tot = sc[:, 4:5]
        nc.vector.reciprocal(sc[:, 7:8], tot)
        nc.vector.tensor_scalar_mul(dst[:, m, 0:N], ex[:, 0:N], sc[:, 7:8])

    def body():
        _emit_body(nc, tc, main, psum, smx, evq, gemm, lhsT_of, rhs_of,
                   copy_evict, w_copy, w_copy_off, w_bias_act, fm_transpose,
                   softmax_rows, adj_sb, x_sb, eps_sb, We1, Kemb1, Kpool1,
                   We2, Kemb2, Kpool2, We3, Wd0, Wd1, Wd2, Wdf,
                   be1, be2, be3, bd0, bd1, bd2, bdf, out_d, mean_d, lv_d)

    if chain > 1:
        with tc.For_i(0, chain, 1):
            body()
    else:
        body()

    smx.release()
    psum.release()
    main.release()


def _emit_body(nc, tc, main, psum, smx, evq, gemm, lhsT_of, rhs_of,
               copy_evict, w_copy, w_copy_off, w_bias_act, fm_transpose,
               softmax_rows, adj_sb, x_sb, eps_sb, We1, Kemb1, Kpool1,
               We2, Kemb2, Kpool2, We3, Wd0, Wd1, Wd2, Wdf,
               be1, be2, be3, bd0, bd1, bd2, bdf, out_d, mean_d, lv_d):
    # ================= encoder stage 1 (2048 nodes) =================
    # fmAx = (adj @ x)^T computed directly feature-major: lhsT=x, rhs=adj
    fmAx = main.tile([P, 1, N0], BF16, tag="g4a")
    nc.vector.memset(fmAx[:], 0.0)
    for n0 in range(0, N0, 512):
        ps = psum.tile([P, 512], F32, tag="ps")
        for ki in range(N0 // P):
            nc.tensor.matmul(ps[0:F, :], x_sb[:, ki, 0:F],
                             adj_sb[:, ki, n0:n0 + 512],
                             start=(ki == 0), stop=(ki == N0 // P - 1))
        copy_evict(fmAx[0:F, 0, n0:n0 + 512], ps[0:F, 0:512])

    h1 = main.tile([P, N0 // P, H], BF16, tag="g8b")
    gemm(N0, P, H, lhsT_of(fmAx), rhs_of(We1), w_bias_act(h1, be1, AF.Relu))

    Ah1 = main.tile([P, N0 // P, H], BF16, tag="g8a")
    gemm(N0, N0, H, lhsT_of(adj_sb), rhs_of(h1), w_copy(Ah1))

    fmAh1 = main.tile([P, H // P, N0], BF16, tag="g8b")
    fm_transpose(fmAh1, Ah1, N0, H)

    s1 = main.tile([P, N0 // P, N1], BF16, tag="s1")
    for m in range(N0 // P):
        softmax_rows(s1, m, N1, lhsT_of(fmAh1), rhs_of(Kpool1), H // P)

    p1a = main.tile([P, N1 // P, H], BF16, tag="g4a")
    gemm(N1, N0, H, lhsT_of(s1), rhs_of(Ah1), w_copy(p1a))

    fmp1a = main.tile([P, H // P, N1], BF16, tag="g4b")
    fm_transpose(fmp1a, p1a, N1, H)

    fmp1 = main.tile([P, H // P, N1], BF16, tag="g4a")
    gemm(H, H, N1, lhsT_of(Kemb1), rhs_of(fmp1a), w_copy(fmp1))

    # A1 = s1^T (adj @ s1), computed in two 512-wide column chunks to halve
    # the AS1 buffer (16K instead of 32K per partition)
    A1 = main.tile([P, N1 // P, N1], BF16, tag="A1")
    for nch in range(2):
        AS1h = main.tile([P, N0 // P, 512], BF16, tag="AS1h")
        gemm(N0, N0, 512, lhsT_of(adj_sb), rhs_of(s1, off=nch * 512),
             w_copy(AS1h))
        gemm(N1, N0, 512, lhsT_of(s1), rhs_of(AS1h),
             w_copy_off(A1, nch * 512))

    # ================= encoder stage 2 (1024 nodes) =================
    xW2 = main.tile([P, N1 // P, H], BF16, tag="g4c")
    gemm(N1, H, H, lhsT_of(fmp1), rhs_of(We2), w_copy(xW2))

    h2 = main.tile([P, N1 // P, H], BF16, tag="g4d")
    gemm(N1, N1, H, lhsT_of(A1), rhs_of(xW2), w_bias_act(h2, be2, AF.Relu))

    Ah2 = main.tile([P, N1 // P, H], BF16, tag="g4a")
    gemm(N1, N1, H, lhsT_of(A1), rhs_of(h2), w_copy(Ah2))

    fmAh2 = main.tile([P, H // P, N1], BF16, tag="g4b")
    fm_transpose(fmAh2, Ah2, N1, H)

    s2 = main.tile([P, N1 // P, N2], BF16, tag="g8a")
    for m in range(N1 // P):
        softmax_rows(s2, m, N2, lhsT_of(fmAh2), rhs_of(Kpool2), H // P)

    p2a = main.tile([P, N2 // P, H], BF16, tag="g4c")
    gemm(N2, N1, H, lhsT_of(s2), rhs_of(Ah2), w_copy(p2a))

    fmp2a = main.tile([P, H // P, N2], BF16, tag="g4d")
    fm_transpose(fmp2a, p2a, N2, H)

    fmp2 = main.tile([P, H // P, N2], BF16, tag="g2")
    gemm(H, H, N2, lhsT_of(Kemb2), rhs_of(fmp2a), w_copy(fmp2))

    AS2 = main.tile([P, N1 // P, N2], BF16, tag="g8c")
    gemm(N1, N1, N2, lhsT_of(A1), rhs_of(s2), w_copy(AS2))

    A2 = main.tile([P, N2 // P, N2], BF16, tag="g4a")
    gemm(N2, N1, N2, lhsT_of(s2), rhs_of(AS2), w_copy(A2))

    # ================= latent head =================
    xW3 = main.tile([P, N2 // P, 2 * LAT], BF16, tag="g4c")
    gemm(N2, H, 2 * LAT, lhsT_of(fmp2), rhs_of(We3), w_copy(xW3))

    h3f = main.tile([P, N2 // P, 2 * LAT], F32, tag="g4d")
    gemm(N2, N2, 2 * LAT, lhsT_of(A2), rhs_of(xW3), w_bias_act(h3f, be3, None))

    nc.sync.dma_start(out=_r3(mean_d), in_=h3f[:, :, 0:LAT])
    nc.sync.dma_start(out=_r3(lv_d), in_=h3f[:, :, LAT:2 * LAT])

    zstd = main.tile([P, N2 // P, LAT], F32, tag="g2")
    z = main.tile([P, N2 // P, LAT], BF16, tag="z")
    nc.scalar.activation(zstd[:], h3f[:, :, LAT:2 * LAT], AF.Exp, scale=0.5)
    nc.vector.tensor_tensor(zstd[:], zstd[:], eps_sb[:], ALU.mult)
    nc.vector.tensor_tensor(z[:], zstd[:], h3f[:, :, 0:LAT], ALU.add)

    # ================= decoder =================
    # fmA2z = (A2 @ z)^T directly feature-major: lhsT=z, rhs=A2
    fmA2z = main.tile([P, 1, N2], BF16, tag="g4d")
    nc.vector.memset(fmA2z[:], 0.0)
    ps = psum.tile([P, 512], F32, tag="ps")
    for ki in range(N2 // P):
        nc.tensor.matmul(ps[0:LAT, :], z[:, ki, 0:LAT], A2[:, ki, 0:N2],
                         start=(ki == 0), stop=(ki == N2 // P - 1))
    copy_evict(fmA2z[0:LAT, 0, 0:N2], ps[0:LAT, 0:N2])

    d0 = main.tile([P, N2 // P, H], BF16, tag="g4c")
    gemm(N2, P, H, lhsT_of(fmA2z), rhs_of(Wd0), w_bias_act(d0, bd0, AF.Relu))

    # fmu1 = (s2 @ d0)^T : lhsT = d0 (pool nodes on partitions), rhs = s2^T
    # streamed via DMA-XBAR transposes of the resident s2
    fmu1 = main.tile([P, H // P, N1], BF16, tag="g4b")
    for n0 in range(0, N1, 512):
        pss = [psum.tile([P, 512], F32, tag="ps", name=f"ps_u{n0}_{i}") for i in range(H // P)]
        for ki in range(N2 // P):
            st = main.tile([P, 512], BF16, tag="strm", bufs=3)
            for j in range(4):
                nc.sync.dma_start(out=st[:, j * P:(j + 1) * P],
                                  in_=s2[:, n0 // P + j, ki * P:(ki + 1) * P],
                                  transpose=True)
            for m in range(H // P):
                nc.tensor.matmul(pss[m][:, :], d0[:, ki, m * P:(m + 1) * P],
                                 st[:, :], start=(ki == 0),
                                 stop=(ki == N2 // P - 1))
        for m in range(H // P):
            copy_evict(fmu1[:, m, n0:n0 + 512], pss[m][:, 0:512])

    xWd1 = main.tile([P, N1 // P, H], BF16, tag="g4c")
    gemm(N1, H, H, lhsT_of(fmu1), rhs_of(Wd1), w_copy(xWd1))

    d1 = main.tile([P, N1 // P, H], BF16, tag="g4a")
    gemm(N1, N1, H, lhsT_of(A1), rhs_of(xWd1), w_bias_act(d1, bd1, AF.Relu))

    # fmu2 = (s1 @ d1)^T : lhsT = d1, rhs = s1^T streamed via DMA transposes
    fmu2 = main.tile([P, H // P, N0], BF16, tag="g8b")
    for n0 in range(0, N0, 512):
        pss = [psum.tile([P, 512], F32, tag="ps", name=f"ps_u{n0}_{i}") for i in range(H // P)]
        for ki in range(N1 // P):
            st = main.tile([P, 512], BF16, tag="strm", bufs=3)
            for j in range(4):
                nc.sync.dma_start(out=st[:, j * P:(j + 1) * P],
                                  in_=s1[:, n0 // P + j, ki * P:(ki + 1) * P],
                                  transpose=True)
            for m in range(H // P):
                nc.tensor.matmul(pss[m][:, :], d1[:, ki, m * P:(m + 1) * P],
                                 st[:, :], start=(ki == 0),
                                 stop=(ki == N1 // P - 1))
        for m in range(H // P):
            copy_evict(fmu2[:, m, n0:n0 + 512], pss[m][:, 0:512])

    xWd2 = main.tile([P, N0 // P, H], BF16, tag="g8c")
    gemm(N0, H, H, lhsT_of(fmu2), rhs_of(Wd2), w_copy(xWd2))

    d2 = main.tile([P, N0 // P, H], BF16, tag="g8a")
    gemm(N0, N0, H, lhsT_of(adj_sb), rhs_of(xWd2), w_bias_act(d2, bd2, AF.Relu))

    fmd2 = main.tile([P, H // P, N0], BF16, tag="g8c")
    fm_transpose(fmd2, d2, N0, H)

    xWdf = main.tile([P, N0 // P, F], BF16, tag="g4c")
    gemm(N0, H, F, lhsT_of(fmd2), rhs_of(Wdf), w_copy(xWdf))

    outf = main.tile([P, N0 // P, F], F32, tag="outf")

    def w_out(m, n0, w, ps):
        # softplus(x) = ln(1 + exp(x)); pre-activations here are O(+-10) so
        # the direct form is stable in f32 (no Softplus ACT table on TRN2)
        o = outf[:, m, n0:n0 + w]
        t = main.tile([P, F], F32, tag="sp", bufs=2, name=f"sp{m}")
        nc.vector.tensor_tensor(o, ps[:, 0:w], bdf[:, n0:n0 + w], ALU.add)
        nc.scalar.activation(t[:, 0:w], o, AF.Exp)
        nc.vector.tensor_scalar_add(t[:, 0:w], t[:, 0:w], 1.0)
        nc.scalar.activation(o, t[:, 0:w], AF.Ln)

    gemm(N0, N0, F, lhsT_of(adj_sb), rhs_of(xWdf), w_out)
    nc.sync.dma_start(out=_r3(out_d), in_=outf[:])


def get_nc(chain=1):
    global _CACHED_NC
    if _CACHED_NC is None:
        _CACHED_NC = {}
    if chain not in _CACHED_NC:
        _CACHED_NC[chain] = _build(chain)
    return _CACHED_NC[chain]


def make_in_maps(inputs):
    bf = ml_dtypes.bfloat16
    adj_bf = np.ascontiguousarray(inputs["adj"]).astype(bf)
    x_bf = np.ascontiguousarray(inputs["x"]).astype(bf)
    eps_f = np.ascontiguousarray(inputs["eps"]).astype(np.float32)
    wnames = ["We1", "Kemb1", "Kpool1", "We2", "Kemb2", "Kpool2", "We3",
              "Wd0", "Wd1", "Wd2", "Wdf"]
    bnames = ["be1", "be2", "be3", "bd0", "bd1", "bd2", "bdf"]
    wvals = {n: np.ascontiguousarray(inputs[n]).astype(bf) for n in wnames}
    bvals = {n: np.ascontiguousarray(inputs[n]).astype(np.float32) for n in bnames}
    in_maps = []
    for c in range(NCORES):
        m = {"x": x_bf[c], "eps": eps_f[c], "adj": adj_bf}
        m.update(wvals)
        m.update(bvals)
        in_maps.append(m)
    return in_maps


def kernel(**inputs):
    from concourse.bass_utils import run_bass_kernel_spmd

    nc = get_nc()
    in_maps = make_in_maps(inputs)
    res = run_bass_kernel_spmd(nc, in_maps, core_ids=list(range(NCORES)))
    out = np.stack([res.results[c]["out"] for c in range(NCORES)]).astype(np.float32)
    mean = np.stack([res.results[c]["mean"] for c in range(NCORES)]).astype(np.float32)
    lv = np.stack([res.results[c]["log_var"] for c in range(NCORES)]).astype(np.float32)
    return out, mean, lv


# revision 25
# speedup vs baseline: 1.7825x; 1.7825x over previous
"""Batch Graph VAE (GCN + DiffPool encoder, DiffUnpool decoder) on 8 TRN2 NeuronCores.

Sharding: data-parallel over batch (B=8 -> 1 sample/core); adj + weights replicated.

Per-core math (restructured with matmul associativity; adj/A1/A2 symmetric so they
are used directly as the TensorE stationary operand lhsT):
  fmAx = (adj @ x)^T                  h1 = relu(Ax @ We1 + be1)
  Ah1  = adj @ h1                     s1 = softmax(Ah1 @ Kpool1)
  p1   = (s1^T Ah1) @ Kemb1           A1 = s1^T (adj @ s1)
  h2   = relu(A1 @ (p1 We2) + be2)    Ah2 = A1 @ h2
  s2   = softmax(Ah2 @ Kpool2)        p2 = (s2^T Ah2) @ Kemb2
  A2   = s2^T (A1 @ s2)               h3 = A2 @ (p2 We3) + be3
  mean, lv = split(h3); z = mean + exp(.5 lv) eps
  d0   = relu((A2 @ z) @ Wd0 + bd0)   u1 = s2 @ d0
  d1   = relu(A1 @ (u1 Wd1) + bd1)    u2 = s1 @ d1
  d2   = relu(adj @ (u2 Wd2) + bd2)   out = softplus(adj @ (d2 Wdf) + bdf)

All matmuls in bf16 with fp32 PSUM accumulation (measured 6.7e-3 rel err vs fp32
reference on the host; fp32 everywhere would be 4x slower on the PE).
Feature-major ("fm") copies needed for X@W products come from DMA-XBAR transposes
(bf16 128x128 tiles) so the PE does no transpose work. SBUF slot groups share
tags (serial lifetimes) to fit the ~208KB/partition budget.
"""

import numpy as np
import ml_dtypes

import concourse.bass as bass
import concourse.mybir as mybir
import concourse.tile as tile
from concourse import bacc
from concourse.masks import make_identity

dt = mybir.dt
F32 = dt.float32
BF16 = dt.bfloat16
AF = mybir.ActivationFunctionType
ALU = mybir.AluOpType
AX = mybir.AxisListType

P = 128
N0, N1, N2 = 2048, 1024, 512
F, H, LAT = 64, 256, 64
NCORES = 8

_CACHED_NC = None

# diagnostic knob: replace all DMA-XBAR transposes with plain DMA copies
# (numerically WRONG - timing isolation only)
DIAG_NO_DMAT = False


def _r3(d):
    # DRAM [nodes, f] -> [p, nodes/P, f] with node = o*P + p
    return d[:].rearrange("(o p) f -> p o f", p=P)


def _build(chain=1):
    nc = bacc.Bacc(trn_type="TRN2", target_bir_lowering=False)

    # ---- DRAM I/O ----
    x_d = nc.dram_tensor("x", [N0, F], BF16, kind="ExternalInput")
    eps_d = nc.dram_tensor("eps", [N2, LAT], F32, kind="ExternalInput")
    adj_d = nc.dram_tensor("adj", [N0, N0], BF16, kind="ExternalInput")
    W = {}
    for name, shape in [
        ("We1", [F, H]), ("Kemb1", [H, H]), ("Kpool1", [H, N1]),
        ("We2", [H, H]), ("Kemb2", [H, H]), ("Kpool2", [H, N2]),
        ("We3", [H, 2 * LAT]), ("Wd0", [LAT, H]), ("Wd1", [H, H]),
        ("Wd2", [H, H]), ("Wdf", [H, F]),
    ]:
        W[name] = nc.dram_tensor(name, shape, BF16, kind="ExternalInput")
    Bs = {}
    for name, n in [
        ("be1", H), ("be2", H), ("be3", 2 * LAT),
        ("bd0", H), ("bd1", H), ("bd2", H), ("bdf", F),
    ]:
        Bs[name] = nc.dram_tensor(name, [n], F32, kind="ExternalInput")
    # feature-major output [F, N0]; host transposes (free on device:
    # bias becomes per-partition and fuses into the ACT Exp of softplus)
    out_d = nc.dram_tensor("out", [F, N0], F32, kind="ExternalOutput")
    mean_d = nc.dram_tensor("mean", [N2, LAT], F32, kind="ExternalOutput")
    lv_d = nc.dram_tensor("log_var", [N2, LAT], F32, kind="ExternalOutput")

    with tile.TileContext(nc) as tc:
        _emit(nc, tc, x_d, eps_d, adj_d, W, Bs, out_d, mean_d, lv_d, chain)

    nc.finalize()
    return nc


def _emit(nc, tc, x_d, eps_d, adj_d, W, Bs, out_d, mean_d, lv_d, chain=1):
    main = tc.alloc_tile_pool(name="main", bufs=1)
    psum = tc.alloc_tile_pool(name="psum", bufs=8, space="PSUM")
    smx = tc.alloc_tile_pool(name="smx", bufs=2)
    evq = [0]

    # ---------- constants / weights ----------
    adj_sb = main.tile([P, N0 // P, N0], BF16, tag="adj")
    adj_r = _r3(adj_d)
    for o in range(N0 // P):
        nc.sync.dma_start(out=adj_sb[:, o, :], in_=adj_r[:, o, :])

    def load_w(name, k, n):
        t = main.tile([P, k // P, n], BF16, tag=name)
        nc.sync.dma_start(out=t[:], in_=_r3(W[name]))
        return t

    def load_w_pad(name, k, n):
        # contraction dim k < 128: zero-pad partitions [k:128]
        t = main.tile([P, 1, n], BF16, tag=name)
        nc.vector.memset(t[:], 0.0)
        nc.sync.dma_start(out=t[0:k, 0, :], in_=W[name][:, :])
        return t

    We1 = load_w_pad("We1", F, H)
    Kemb1 = load_w("Kemb1", H, H)
    Kpool1 = load_w("Kpool1", H, N1)
    We2 = load_w("We2", H, H)
    Kemb2 = load_w("Kemb2", H, H)
    Kpool2 = load_w("Kpool2", H, N2)
    We3 = load_w("We3", H, 2 * LAT)
    Wd0 = load_w_pad("Wd0", LAT, H)
    Wd1 = load_w("Wd1", H, H)
    Wd2 = load_w("Wd2", H, H)
    Wdf = load_w("Wdf", H, F)

    def load_bias(name, n):
        t = main.tile([P, n], F32, tag=name)
        src = Bs[name][:]
        bcast = bass.AP(tensor=src.tensor, offset=src.offset,
                        ap=[[0, P], list(src.ap[0])])
        nc.gpsimd.dma_start(out=t[:], in_=bcast)
        return t

    be1 = load_bias("be1", H)
    be2 = load_bias("be2", H)
    be3 = load_bias("be3", 2 * LAT)
    bd0 = load_bias("bd0", H)
    bd1 = load_bias("bd1", H)
    bd2 = load_bias("bd2", H)
    bdf = load_bias("bdf", F)

    eps_sb = main.tile([P, N2 // P, LAT], F32, tag="eps")
    nc.sync.dma_start(out=eps_sb[:], in_=_r3(eps_d))

    ident = main.tile([P, P], BF16, tag="ident")
    make_identity(nc, ident)

    # bd2 in per-partition layout (for the fused ACT bias+relu on fm(d2))
    bd2p = main.tile([P, H // P], F32, tag="bd2p")
    nc.sync.dma_start(out=bd2p[:], in_=Bs["bd2"][:].rearrange("(o p) -> p o", p=P))

    # bdf per-partition for the feature-major output layer
    bdfp = main.tile([P, 1], F32, tag="bdfp")
    nc.vector.memset(bdfp[:], 0.0)
    nc.sync.dma_start(out=bdfp[0:F, 0], in_=Bs["bdf"][:])

    x_sb = main.tile([P, N0 // P, F], BF16, tag="x")
    nc.sync.dma_start(out=x_sb[:], in_=_r3(x_d))

    # ---------- helpers ----------
    def lhsT_of(t):
        return lambda ki, m: t[:, ki, m * P:(m + 1) * P]

    def rhs_of(t, off=0):
        return lambda ki, n0, w: t[:, ki, off + n0:off + n0 + w]

    def gemm(M, K, N, lhsT_fn, rhs_fn, writer):
        mt, kt = M // P, (K + P - 1) // P
        for m in range(mt):
            for n0 in range(0, N, 512):
                w = min(512, N - n0)
                ps = psum.tile([P, 512], F32, tag="ps")
                for ki in range(kt):
                    nc.tensor.matmul(ps[:, 0:w], lhsT_fn(ki, m),
                                     rhs_fn(ki, n0, w),
                                     start=(ki == 0), stop=(ki == kt - 1))
                writer(m, n0, w, ps)

    def copy_evict(dst_ap, src_ap):
        if evq[0] % 2 == 0:
            nc.vector.tensor_copy(out=dst_ap, in_=src_ap)
        else:
            nc.scalar.copy(out=dst_ap, in_=src_ap)
        evq[0] += 1

    def w_copy(dst):
        def wr(m, n0, w, ps):
            copy_evict(dst[:, m, n0:n0 + w], ps[:, 0:w])
        return wr

    def w_copy_off(dst, off):
        def wr(m, n0, w, ps):
            copy_evict(dst[:, m, off + n0:off + n0 + w], ps[:, 0:w])
        return wr

    def w_bias_act(dst, bias_t, act):
        def wr(m, n0, w, ps):
            o = dst[:, m, n0:n0 + w]
            nc.vector.tensor_tensor(o, ps[:, 0:w], bias_t[:, n0:n0 + w], ALU.add)
            if act is not None:
                nc.scalar.activation(o, o, act)
        return wr

    def fm_transpose(dst_fm, src_nat, nodes, feat):
        # DMA-XBAR transpose of bf16 128x128 tiles:
        # src_nat [P, nodes/P, feat] -> dst_fm [P, feat/P, nodes]
        for c in range(feat // P):
            for r in range(nodes // P):
                nc.sync.dma_start(out=dst_fm[:, c, r * P:(r + 1) * P],
                                  in_=src_nat[:, r, c * P:(c + 1) * P],
                                  transpose=not DIAG_NO_DMAT)

    def softmax_rows(dst, m, N, lhsT_fn, rhs_fn, kt):
        # pre-activation row-tile m of shape [P, N]; softmax over free axis
        nch = (N + 511) // 512
        pss = []
        for n0 in range(0, N, 512):
            ps = psum.tile([P, 512], F32, tag="ps")
            for ki in range(kt):
                nc.tensor.matmul(ps[:, :], lhsT_fn(ki, m), rhs_fn(ki, n0, 512),
                                 start=(ki == 0), stop=(ki == kt - 1))
            pss.append(ps)
        sc = smx.tile([P, 8], F32, tag="sstat")
        for i, ps in enumerate(pss):
            nc.vector.reduce_max(sc[:, i:i + 1], ps[:, :], axis=AX.X)
        if nch == 2:
            nc.vector.tensor_tensor(sc[:, 2:3], sc[:, 0:1], sc[:, 1:2], ALU.max)
            mx = sc[:, 2:3]
        else:
            mx = sc[:, 0:1]
        nc.vector.tensor_scalar_mul(sc[:, 3:4], mx, -1.0)
        # exp straight into the (bf16) destination, then normalize in place
        for i, ps in enumerate(pss):
            nc.scalar.activation(dst[:, m, i * 512:(i + 1) * 512], ps[:, :],
                                 AF.Exp, bias=sc[:, 3:4],
                                 accum_out=sc[:, 4 + i:5 + i])
        if nch == 2:
            nc.vector.tensor_tensor(sc[:, 6:7], sc[:, 4:5], sc[:, 5:6], ALU.add)
            tot = sc[:, 6:7]
        else:
            tot = sc[:, 4:5]
        nc.vector.reciprocal(sc[:, 7:8], tot)
        nc.vector.tensor_scalar_mul(dst[:, m, 0:N], dst[:, m, 0:N], sc[:, 7:8])

    def body():
        _emit_body(nc, tc, main, psum, smx, evq, gemm, lhsT_of, rhs_of,
                   copy_evict, w_copy, w_copy_off, w_bias_act, fm_transpose,
                   softmax_rows, adj_sb, x_sb, eps_sb, We1, Kemb1, Kpool1,
                   We2, Kemb2, Kpool2, We3, Wd0, Wd1, Wd2, Wdf,
                   be1, be2, be3, bd0, bd1, bd2, bdf, out_d, mean_d, lv_d,
                   ident, bd2p, bdfp)

    if chain > 1:
        with tc.For_i(0, chain, 1):
            body()
    else:
        body()

    smx.release()
    psum.release()
    main.release()


def _emit_body(nc, tc, main, psum, smx, evq, gemm, lhsT_of, rhs_of,
               copy_evict, w_copy, w_copy_off, w_bias_act, fm_transpose,
               softmax_rows, adj_sb, x_sb, eps_sb, We1, Kemb1, Kpool1,
               We2, Kemb2, Kpool2, We3, Wd0, Wd1, Wd2, Wdf,
               be1, be2, be3, bd0, bd1, bd2, bdf, out_d, mean_d, lv_d,
               ident, bd2p, bdfp):
    # ================= encoder stage 1 (2048 nodes) =================
    # fmAx = (adj @ x)^T computed directly feature-major: lhsT=x, rhs=adj
    fmAx = main.tile([P, 1, N0], BF16, tag="g4a")
    nc.vector.memset(fmAx[:], 0.0)
    for n0 in range(0, N0, 512):
        ps = psum.tile([P, 512], F32, tag="ps")
        for ki in range(N0 // P):
            nc.tensor.matmul(ps[0:F, :], x_sb[:, ki, 0:F],
                             adj_sb[:, ki, n0:n0 + 512],
                             start=(ki == 0), stop=(ki == N0 // P - 1))
        copy_evict(fmAx[0:F, 0, n0:n0 + 512], ps[0:F, 0:512])

    h1 = main.tile([P, N0 // P, H], BF16, tag="g8b")
    gemm(N0, P, H, lhsT_of(fmAx), rhs_of(We1), w_bias_act(h1, be1, AF.Relu))

    # fmAh1 = (adj @ h1)^T directly feature-major: lhsT=h1, rhs=adj
    fmAh1 = main.tile([P, H // P, N0], BF16, tag="g8d")
    gemm(H, N0, N0, lhsT_of(h1), rhs_of(adj_sb), w_copy(fmAh1))

    s1 = main.tile([P, N0 // P, N1], BF16, tag="s1")
    for m in range(N0 // P):
        softmax_rows(s1, m, N1, lhsT_of(fmAh1), rhs_of(Kpool1), H // P)

    # p1 = (s1^T Ah1) Kemb1 = ((Ah1 Kemb1)^T s1)^T -> fm(p1) directly:
    # AhK1 = Ah1 @ Kemb1 (nat, from fm(Ah1));  fm(p1) = matmul(lhsT=AhK1, rhs=s1)
    AhK1 = main.tile([P, N0 // P, H], BF16, tag="g8a")
    gemm(N0, H, H, lhsT_of(fmAh1), rhs_of(Kemb1), w_copy(AhK1))

    fmp1 = main.tile([P, H // P, N1], BF16, tag="g4a")
    gemm(H, N0, N1, lhsT_of(AhK1), rhs_of(s1), w_copy(fmp1))

    # A1 = s1^T (adj @ s1), computed in two 512-wide column chunks to halve
    # the AS1 buffer (16K instead of 32K per partition). A1 is symmetric
    # (adj is), so strictly-lower-triangle 128x256 blocks are skipped and
    # filled by transposing the mirrored upper blocks (identity matmuls).
    A1 = main.tile([P, N1 // P, N1], BF16, tag="A1")
    fills = []
    for nch in range(2):
        AS1h = main.tile([P, N0 // P, 512], BF16, tag="AS1h")
        gemm(N0, N0, 512, lhsT_of(adj_sb), rhs_of(s1, off=nch * 512),
             w_copy(AS1h))
        for m in range(N1 // P):
            for nn in range(2):
                col0 = nch * 512 + nn * 256
                if col0 + 256 <= m * P:
                    fills.append((m, col0))
                    continue
                ps = psum.tile([P, 512], F32, tag="ps",
                               name=f"ps_a1_{nch}_{m}_{nn}")
                for ki in range(N0 // P):
                    nc.tensor.matmul(ps[:, 0:256],
                                     s1[:, ki, m * P:(m + 1) * P],
                                     AS1h[:, ki, nn * 256:nn * 256 + 256],
                                     start=(ki == 0), stop=(ki == N0 // P - 1))
                copy_evict(A1[:, m, col0:col0 + 256], ps[:, 0:256])
    for m, col0 in fills:
        # A1[m-rows, col0:col0+256] = A1[col0:col0+256 rows, m-cols]^T
        ps = psum.tile([P, 512], F32, tag="ps", name=f"ps_a1f_{m}_{col0}")
        for j in range(2):
            nc.tensor.matmul(ps[:, j * P:(j + 1) * P],
                             A1[:, col0 // P + j, m * P:(m + 1) * P],
                             ident[:], start=True, stop=True)
        copy_evict(A1[:, m, col0:col0 + 256], ps[:, 0:256])

    # ================= encoder stage 2 (1024 nodes) =================
    xW2 = main.tile([P, N1 // P, H], BF16, tag="g4c")
    gemm(N1, H, H, lhsT_of(fmp1), rhs_of(We2), w_copy(xW2))

    h2 = main.tile([P, N1 // P, H], BF16, tag="g4b")
    gemm(N1, N1, H, lhsT_of(A1), rhs_of(xW2), w_bias_act(h2, be2, AF.Relu))

    # fmAh2 = (A1 @ h2)^T directly feature-major: lhsT=h2, rhs=A1
    fmAh2 = main.tile([P, H // P, N1], BF16, tag="g4a")
    gemm(H, N1, N1, lhsT_of(h2), rhs_of(A1), w_copy(fmAh2))

    s2 = main.tile([P, N1 // P, N2], BF16, tag="g8a")
    for m in range(N1 // P):
        softmax_rows(s2, m, N2, lhsT_of(fmAh2), rhs_of(Kpool2), H // P)

    AhK2 = main.tile([P, N1 // P, H], BF16, tag="g4b")
    gemm(N1, H, H, lhsT_of(fmAh2), rhs_of(Kemb2), w_copy(AhK2))

    fmp2 = main.tile([P, H // P, N2], BF16, tag="g2")
    gemm(H, N1, N2, lhsT_of(AhK2), rhs_of(s2), w_copy(fmp2))

    AS2 = main.tile([P, N1 // P, N2], BF16, tag="g8c")
    gemm(N1, N1, N2, lhsT_of(A1), rhs_of(s2), w_copy(AS2))

    A2 = main.tile([P, N2 // P, N2], BF16, tag="g4a")
    gemm(N2, N1, N2, lhsT_of(s2), rhs_of(AS2), w_copy(A2))

    # ================= latent head =================
    xW3 = main.tile([P, N2 // P, 2 * LAT], BF16, tag="g4c")
    gemm(N2, H, 2 * LAT, lhsT_of(fmp2), rhs_of(We3), w_copy(xW3))

    h3f = main.tile([P, N2 // P, 2 * LAT], F32, tag="g4d")
    gemm(N2, N2, 2 * LAT, lhsT_of(A2), rhs_of(xW3), w_bias_act(h3f, be3, None))

    nc.sync.dma_start(out=_r3(mean_d), in_=h3f[:, :, 0:LAT])
    nc.sync.dma_start(out=_r3(lv_d), in_=h3f[:, :, LAT:2 * LAT])

    zstd = main.tile([P, N2 // P, LAT], F32, tag="g2")
    z = main.tile([P, N2 // P, LAT], BF16, tag="z")
    nc.scalar.activation(zstd[:], h3f[:, :, LAT:2 * LAT], AF.Exp, scale=0.5)
    nc.vector.tensor_tensor(zstd[:], zstd[:], eps_sb[:], ALU.mult)
    nc.vector.tensor_tensor(z[:], zstd[:], h3f[:, :, 0:LAT], ALU.add)

    # ================= decoder =================
    # fmA2z = (A2 @ z)^T directly feature-major: lhsT=z, rhs=A2
    fmA2z = main.tile([P, 1, N2], BF16, tag="g4d")
    nc.vector.memset(fmA2z[:], 0.0)
    ps = psum.tile([P, 512], F32, tag="ps")
    for ki in range(N2 // P):
        nc.tensor.matmul(ps[0:LAT, :], z[:, ki, 0:LAT], A2[:, ki, 0:N2],
                         start=(ki == 0), stop=(ki == N2 // P - 1))
    copy_evict(fmA2z[0:LAT, 0, 0:N2], ps[0:LAT, 0:N2])

    d0 = main.tile([P, N2 // P, H], BF16, tag="g4c")
    gemm(N2, P, H, lhsT_of(fmA2z), rhs_of(Wd0), w_bias_act(d0, bd0, AF.Relu))

    # s^T tiles produced on the PE as identity matmuls (out = lhsT^T @ I),
    # streamed through a small pool; each feeds the 2 m-tiles of the unpool.
    def strm_T(src, o_base, ki, nm):
        st = main.tile([P, 512], BF16, tag="strm", bufs=4, name=nm)
        pst = psum.tile([P, 512], F32, tag="ps", name=nm + "_ps")
        for j in range(4):
            nc.tensor.matmul(pst[:, j * P:(j + 1) * P],
                             src[:, o_base + j, ki * P:(ki + 1) * P],
                             ident[:], start=True, stop=True)
        copy_evict(st[:], pst[:])
        return st

    # fmu1 = (s2 @ d0)^T : lhsT = d0 (pool nodes on partitions), rhs = s2^T
    fmu1 = main.tile([P, H // P, N1], BF16, tag="g4b")
    for n0 in range(0, N1, 512):
        pss = [psum.tile([P, 512], F32, tag="ps", name=f"ps_u1_{n0}_{i}")
               for i in range(H // P)]
        for ki in range(N2 // P):
            st = strm_T(s2, n0 // P, ki, f"st1_{n0}_{ki}")
            for m in range(H // P):
                nc.tensor.matmul(pss[m][:, :], d0[:, ki, m * P:(m + 1) * P],
                                 st[:, :], start=(ki == 0),
                                 stop=(ki == N2 // P - 1))
        for m in range(H // P):
            copy_evict(fmu1[:, m, n0:n0 + 512], pss[m][:, 0:512])

    xWd1 = main.tile([P, N1 // P, H], BF16, tag="g4c")
    gemm(N1, H, H, lhsT_of(fmu1), rhs_of(Wd1), w_copy(xWd1))

    d1 = main.tile([P, N1 // P, H], BF16, tag="g4a")
    gemm(N1, N1, H, lhsT_of(A1), rhs_of(xWd1), w_bias_act(d1, bd1, AF.Relu))

    # fmu2 = (s1 @ d1)^T : lhsT = d1, rhs = s1^T streamed via identity matmuls
    fmu2 = main.tile([P, H // P, N0], BF16, tag="g8b")
    for n0 in range(0, N0, 512):
        pss = [psum.tile([P, 512], F32, tag="ps", name=f"ps_u2_{n0}_{i}")
               for i in range(H // P)]
        for ki in range(N1 // P):
            st = strm_T(s1, n0 // P, ki, f"st2_{n0}_{ki}")
            for m in range(H // P):
                nc.tensor.matmul(pss[m][:, :], d1[:, ki, m * P:(m + 1) * P],
                                 st[:, :], start=(ki == 0),
                                 stop=(ki == N1 // P - 1))
        for m in range(H // P):
            copy_evict(fmu2[:, m, n0:n0 + 512], pss[m][:, 0:512])

    xWd2 = main.tile([P, N0 // P, H], BF16, tag="g8d")
    gemm(N0, H, H, lhsT_of(fmu2), rhs_of(Wd2), w_copy(xWd2))

    # fm(d2) = relu((adj @ xWd2 + bd2)^T) directly feature-major:
    # lhsT=xWd2, rhs=adj; bias is per-partition in fm layout -> fused ACT evict
    fmd2 = main.tile([P, H // P, N0], BF16, tag="g8a")

    def w_fmd2(m, n0, w, ps):
        nc.scalar.activation(fmd2[:, m, n0:n0 + w], ps[:, 0:w], AF.Relu,
                             bias=bd2p[:, m:m + 1])

    gemm(H, N0, N0, lhsT_of(xWd2), rhs_of(adj_sb), w_fmd2)

    xWdf = main.tile([P, N0 // P, F], BF16, tag="g4c")
    gemm(N0, H, F, lhsT_of(fmd2), rhs_of(Wdf), w_copy(xWdf))

    # out^T = softplus((adj @ xWdf + bdf))^T computed feature-major:
    # lhsT=xWdf, rhs=adj; bias is per-partition -> fused into the ACT Exp.
    # softplus(x) = ln(1 + exp(x)) (no Softplus ACT table on TRN2; pre-acts
    # are O(+-10) so the direct form is stable in f32)
    for n0 in range(0, N0, 512):
        ps = psum.tile([P, 512], F32, tag="ps", name=f"ps_o{n0}")
        for ki in range(N0 // P):
            nc.tensor.matmul(ps[0:F, :], xWdf[:, ki, 0:F],
                             adj_sb[:, ki, n0:n0 + 512],
                             start=(ki == 0), stop=(ki == N0 // P - 1))
        t = main.tile([P, 512], F32, tag="sp", bufs=2, name=f"sp{n0}")
        nc.scalar.activation(t[0:F, :], ps[0:F, :], AF.Exp, bias=bdfp[0:F, 0:1])
        nc.vector.tensor_scalar_add(t[0:F, :], t[0:F, :], 1.0)
        nc.scalar.activation(t[0:F, :], t[0:F, :], AF.Ln)
        nc.sync.dma_start(out=out_d[:, n0:n0 + 512], in_=t[0:F, :])


def get_nc(chain=1):
    global _CACHED_NC
    if _CACHED_NC is None:
        _CACHED_NC = {}
    if chain not in _CACHED_NC:
        _CACHED_NC[chain] = _build(chain)
    return _CACHED_NC[chain]


def make_in_maps(inputs):
    bf = ml_dtypes.bfloat16
    adj_bf = np.ascontiguousarray(inputs["adj"]).astype(bf)
    x_bf = np.ascontiguousarray(inputs["x"]).astype(bf)
    eps_f = np.ascontiguousarray(inputs["eps"]).astype(np.float32)
    wnames = ["We1", "Kemb1", "Kpool1", "We2", "Kemb2", "Kpool2", "We3",
              "Wd0", "Wd1", "Wd2", "Wdf"]
    bnames = ["be1", "be2", "be3", "bd0", "bd1", "bd2", "bdf"]
    wvals = {n: np.ascontiguousarray(inputs[n]).astype(bf) for n in wnames}
    bvals = {n: np.ascontiguousarray(inputs[n]).astype(np.float32) for n in bnames}
    in_maps = []
    for c in range(NCORES):
        m = {"x": x_bf[c], "eps": eps_f[c], "adj": adj_bf}
        m.update(wvals)
        m.update(bvals)
        in_maps.append(m)
    return in_maps


def kernel(**inputs):
    from concourse.bass_utils import run_bass_kernel_spmd

    nc = get_nc()
    in_maps = make_in_maps(inputs)
    res = run_bass_kernel_spmd(nc, in_maps, core_ids=list(range(NCORES)))
    out = np.stack([np.ascontiguousarray(res.results[c]["out"].T)
                    for c in range(NCORES)]).astype(np.float32)
    mean = np.stack([res.results[c]["mean"] for c in range(NCORES)]).astype(np.float32)
    lv = np.stack([res.results[c]["log_var"] for c in range(NCORES)]).astype(np.float32)
    return out, mean, lv
